# revision 1
# baseline (speedup 1.0000x reference)
"""Grouped-query attention (B=2, T=2048, D=1024, 16 Q heads / 4 KV heads) on
8 Trainium2 NeuronCores — zero-collective version.

Sharding: core i handles batch b = i//4 and head-group g = i%4 (query heads
4g..4g+3, KV head g).  Everything is computed in "transposed" layout
(features on partitions, tokens on the free axis).  bf16 operands with fp32
PSUM accumulation.

  Q^T[pair]  [128, 2048]  = (Wq_pair)^T-chunks x^T          (PE, PSUM acc)
  K^T dup    [128, 2048]  rows 0:64 and 64:128 both K^T (so head-pair
                           S-matmuls read aligned partition ranges)
  V natural  [128, 16, 65] via PE transpose of V^T (+ ones column for the
                           softmax denominator trick)
  S^T tiles  [128s, 512q] = K^T-chunk^T Q^T   (scores, transposed)
  P = exp(S^T/8)           ACT, straight from PSUM, no max subtraction
  O^T+Z      [65, 512]    = [V|1]^T P    (row 64 = softmax denominator)
  normalize  O^T *= 1/Z   (DVE recip + gpsimd/Pool partition broadcast)
  partial out^T [1024,2048] = Wo-rows^T concat^T  (row-parallel out-proj)

No device collectives: each core writes its partial out^T (contraction over
its own 256 concat dims) and the HOST sums the 4 partials per batch.  This
keeps every core's NEFF execution completely independent.
"""

import sys

if "/opt/trn_rl_repo" not in sys.path:
    sys.path.insert(0, "/opt/trn_rl_repo")

import numpy as np

import concourse.bass as bass
import concourse.mybir as mybir
import concourse.tile as tile
from concourse import bacc, library_config
from concourse.bass import ds

F32 = mybir.dt.float32
F32R = mybir.dt.float32r
BF16 = mybir.dt.bfloat16

B, T, D = 2, 2048, 1024
H, KVH, HD = 16, 4, 64
G = H // KVH            # 4 query heads per core
GD = G * HD             # 256 query dims per core
NCORES = 8
PB = 128                # partition block (s-chunk size)
QB = 512                # q block (matmul moving free dim)
NQB = T // QB           # 4
NSC = T // PB           # 16
NDC = D // PB           # 8 contraction chunks of D
EXP_GRP = 2             # s-chunk slots per exp() call (2 PSUM banks each)


def _mask_plan(mask2d):
    """Per q-block list of (j, avlo, mode, gix).

    mode 0: fully visible chunk.  mode 1: causal-diagonal chunk (columns
    below avlo are dead, the [avlo, avlo+128) square is multiplied by the
    triangular keep-mask).  mode 2: generic chunk, multiplied by keep-mask
    tile gix.  Chunks whose block is fully masked are dropped.
    """
    causal = np.array_equal(mask2d, np.triu(np.ones((T, T), dtype=bool), k=1))
    plan = []
    gen_tiles = []
    if causal:
        for qb in range(NQB):
            chunks = [(j, 0, 0, -1) for j in range(4 * qb)]
            chunks += [(4 * qb + k, PB * k, 1, -1) for k in range(4)]
            plan.append(chunks)
        kind = "causal"
    elif not mask2d.any():
        plan = [[(j, 0, 0, -1) for j in range(NSC)] for _ in range(NQB)]
        kind = "nomask"
    else:
        for qb in range(NQB):
            chunks = []
            for j in range(NSC):
                sub = mask2d[QB * qb:QB * (qb + 1), PB * j:PB * (j + 1)]
                if sub.all():
                    continue
                if not sub.any():
                    chunks.append((j, 0, 0, -1))
                else:
                    gen_tiles.append(np.ascontiguousarray((~sub).T))
                    chunks.append((j, 0, 2, len(gen_tiles) - 1))
            plan.append(chunks)
        kind = "generic"
    genmask = (
        np.stack(gen_tiles, axis=0).astype(np.float32)
        if gen_tiles else np.zeros((0, PB, QB), np.float32)
    )
    return kind, plan, genmask


def _build(plan, ngen, has_bias):
    nc = bacc.Bacc(
        "TRN2", target_bir_lowering=False, debug=False, num_devices=NCORES
    )

    xt_d = nc.dram_tensor("xt", [PB, NDC * T], BF16, kind="ExternalInput")
    wq_d = nc.dram_tensor("wq", [PB, NDC * GD], BF16, kind="ExternalInput")
    wkv_d = nc.dram_tensor("wkv", [PB, NDC * PB], BF16, kind="ExternalInput")
    wo_d = nc.dram_tensor("wo", [PB, 2 * D], BF16, kind="ExternalInput")
    tri_d = nc.dram_tensor("tri", [PB, PB], BF16, kind="ExternalInput")
    id_d = nc.dram_tensor("ident", [PB, HD], BF16, kind="ExternalInput")
    out_d = nc.dram_tensor("outT", [PB, NDC * T], BF16, kind="ExternalOutput")
    gen_d = None
    if ngen:
        gen_d = nc.dram_tensor("genmask", [ngen, PB, QB], BF16, kind="ExternalInput")
    if has_bias:
        bq_d = nc.dram_tensor("bqp", [PB, 2], F32, kind="ExternalInput")
        bkv_d = nc.dram_tensor("bkvp", [PB, 1], F32, kind="ExternalInput")

    with tile.TileContext(nc) as tc:
        with (
            tc.tile_pool(name="wts", bufs=1) as wpool,
            tc.tile_pool(name="qkv", bufs=1) as qkvpool,
            tc.tile_pool(name="pp", bufs=4) as ppool,
            tc.tile_pool(name="oo", bufs=2) as opool,
            tc.tile_pool(name="zz", bufs=4) as zpool,
        ):
            # ---- constant / weight / activation loads ------------------
            tri = wpool.tile([PB, PB], BF16, tag="tri", name="tri")
            nc.sync.dma_start(tri[:], tri_d[:])
            ident = wpool.tile([PB, HD], BF16, tag="ident", name="ident")
            nc.sync.dma_start(ident[:], id_d[:])
            nc.gpsimd.load_library(library_config.attnmlp)

            # inputs stream on the two HWDGE rings in parallel: xt on the
            # SP (sync) ring, weights on the Activation ring
            wq_sb = wpool.tile([PB, NDC * GD], BF16, tag="wq", name="wq")
            xt = wpool.tile([PB, NDC * T], BF16, tag="xt", name="xt")
            for c in range(NDC):
                nc.sync.dma_start(wq_sb[:, ds(GD * c, GD)], wq_d[:, ds(GD * c, GD)])
                eng = nc.scalar if c % 2 == 0 else nc.sync
                eng.dma_start(xt[:, ds(T * c, T)], xt_d[:, ds(T * c, T)])
            wkv_sb = wpool.tile([PB, NDC * PB], BF16, tag="wkv", name="wkv")
            nc.sync.dma_start(wkv_sb[:], wkv_d[:])
            wo_sb = wpool.tile([PB, 2 * D], BF16, tag="wo", name="wo")
            nc.sync.dma_start(wo_sb[:], wo_d[:])
            if has_bias:
                bq_sb = wpool.tile([PB, 2], F32, tag="bq", name="bq")
                nc.sync.dma_start(bq_sb[:], bq_d[:])
                bkv_sb = wpool.tile([PB, 1], F32, tag="bkv", name="bkv")
                nc.sync.dma_start(bkv_sb[:], bkv_d[:])

            def psum_to_sbuf(dst, src, bias_col):
                if bias_col is not None:
                    nc.vector.tensor_scalar(
                        dst, src, scalar1=bias_col, scalar2=None,
                        op0=mybir.AluOpType.add,
                    )
                else:
                    nc.vector.tensor_copy(dst, src)

            # ---- projections ------------------------------------------
            qt = [
                qkvpool.tile([PB, T], BF16, tag="qt0", name="qt0"),
                qkvpool.tile([PB, T], BF16, tag="qt1", name="qt1"),
            ]
            kvt = qkvpool.tile([PB, T], BF16, tag="kvt", name="kvt")
            vt = qkvpool.tile([PB, NSC, HD + 1], BF16, tag="vt", name="vt")

            with tc.tile_pool(name="pps", bufs=1, space="PSUM") as pps:
                for p in (0, 1):
                    for nqi in range(NQB):
                        q_ps = pps.tile([PB, QB], F32, tag="qps", bufs=6, name="qps")
                        for c in range(NDC):
                            nc.tensor.matmul(
                                q_ps[:],
                                wq_sb[:, ds(GD * c + PB * p, PB)],
                                xt[:, ds(T * c + QB * nqi, QB)],
                                start=(c == 0),
                                stop=(c == NDC - 1),
                            )
                        psum_to_sbuf(
                            qt[p][:, ds(QB * nqi, QB)], q_ps[:],
                            bq_sb[:, ds(p, 1)] if has_bias else None,
                        )
                for nqi in range(NQB):
                    kv_ps = pps.tile([PB, QB], F32, tag="qps", bufs=6, name="qps")
                    for c in range(NDC):
                        nc.tensor.matmul(
                            kv_ps[:],
                            wkv_sb[:, ds(PB * c, PB)],
                            xt[:, ds(T * c + QB * nqi, QB)],
                            start=(c == 0),
                            stop=(c == NDC - 1),
                        )
                    psum_to_sbuf(
                        kvt[:, ds(QB * nqi, QB)], kv_ps[:],
                        bkv_sb[:, 0:1] if has_bias else None,
                    )
                    # V natural tiles for this block's 4 chunks, then
                    # duplicate K^T into rows 64:128 of the block (the
                    # transposes consumed its V^T rows)
                    v_ps = pps.tile([PB, 4, HD], BF16, tag="vps", bufs=2, name="vps")
                    for k4 in range(4):
                        j = 4 * nqi + k4
                        nc.tensor.transpose(
                            v_ps[:, k4, :],
                            kvt[HD:PB, ds(PB * j, PB)],
                            ident[HD:PB, :],
                        )
                    nc.vector.tensor_copy(
                        vt[:, ds(4 * nqi, 4), 0:HD], v_ps[:]
                    )
                    nc.sync.dma_start(
                        kvt[HD:PB, ds(QB * nqi, QB)],
                        kvt[0:HD, ds(QB * nqi, QB)],
                    )
                nc.vector.memset(vt[:, :, HD:HD + 1], 1.0)

            # ---- attention (head pairs p=0,1), with per-q-block KV-proj,
            # ---- V xbar-transpose, and out-proj all interleaved ---------
            cc = [
                qkvpool.tile([PB, T], BF16, tag="cc0", name="cc0"),
                qkvpool.tile([PB, T], BF16, tag="cc1", name="cc1"),
            ]
            with (
                tc.tile_pool(name="spsum", bufs=2, space="PSUM") as spsum,
                tc.tile_pool(name="opsum", bufs=1, space="PSUM") as opsum,
            ):
                for qb in range(NQB):
                    chunks = plan[qb]
                    for p in (0, 1):
                        slots = [(h, ch) for ch in chunks for h in (0, 1)]
                        oacc = [
                            opsum.tile([HD + 1, QB], F32, tag="oacc", bufs=2, name="oacc")
                            for _ in (0, 1)
                        ]
                        n_of = {0: 0, 1: 0}
                        total = {0: len(chunks), 1: len(chunks)}
                        def grp_tail(grp, pg):
                            # tri-mask (DVE) + O-matmuls (PE) for a group
                            for idx, (h, (j, avlo, mode, gix)) in enumerate(grp):
                                if mode == 1:
                                    nc.vector.tensor_mul(
                                        pg[:, idx, ds(avlo, PB)],
                                        pg[:, idx, ds(avlo, PB)],
                                        tri[:],
                                    )
                                elif mode == 2:
                                    gm = ppool.tile(
                                        [PB, QB], BF16, tag="gm", bufs=4, name="gm"
                                    )
                                    nc.sync.dma_start(gm[:], gen_d[gix])
                                    nc.vector.tensor_mul(
                                        pg[:, idx, :], pg[:, idx, :], gm[:]
                                    )
                                nc.tensor.matmul(
                                    oacc[h][:, ds(avlo, QB - avlo)],
                                    vt[:, j, :],
                                    pg[:, idx, ds(avlo, QB - avlo)],
                                    start=(n_of[h] == 0),
                                    stop=(n_of[h] == total[h] - 1),
                                )
                                n_of[h] += 1

                        # software pipeline: S(g+1) is issued on PE before
                        # O(g), so exp(g) overlaps S(g+1) instead of
                        # stalling PE.
                        pending = None
                        for gi in range(0, len(slots), EXP_GRP):
                            grp = slots[gi:gi + EXP_GRP]
                            # both slots of a group share the same chunk,
                            # hence the same live column range [avlo:QB)
                            avg = grp[0][1][1]
                            sg = spsum.tile(
                                [PB, EXP_GRP, QB], F32, tag="sg", bufs=2, name="sg"
                            )
                            pg = ppool.tile(
                                [PB, EXP_GRP, QB], BF16, tag="pg", bufs=4, name="pg"
                            )
                            for idx, (h, (j, avlo, mode, gix)) in enumerate(grp):
                                nc.tensor.matmul(
                                    sg[:, idx, ds(avlo, QB - avlo)],
                                    kvt[ds(HD * h, HD), ds(PB * j, PB)],
                                    qt[p][ds(HD * h, HD), ds(QB * qb + avlo, QB - avlo)],
                                    start=True,
                                    stop=True,
                                )
                            if pending is not None:
                                grp_tail(*pending)
                            nc.scalar.activation(
                                pg[:, 0:len(grp), ds(avg, QB - avg)],
                                sg[:, 0:len(grp), ds(avg, QB - avg)],
                                mybir.ActivationFunctionType.Exp,
                                scale=1.0 / float(np.sqrt(HD)),
                            )
                            pending = (grp, pg)
                        if pending is not None:
                            grp_tail(*pending)
                        # normalize: O^T[h] /= Z[h]   (Z = row 64 of oacc);
                        # 1/Z broadcast across 64 partitions on the (idle)
                        # Pool engine.
                        for h in (0, 1):
                            zr = zpool.tile([1, QB], F32, tag="zr", name="zr")
                            nc.vector.reciprocal(zr[:], oacc[h][HD:HD + 1, :])
                            zb = zpool.tile([HD, QB], F32, tag="zb", bufs=2, name="zb")
                            nc.gpsimd.partition_broadcast(zb[:], zr[:])
                            nc.vector.tensor_mul(
                                cc[p][ds(HD * h, HD), ds(QB * qb, QB)],
                                oacc[h][0:HD, :],
                                zb[:],
                            )
                    # out-proj for this q-block (row-parallel partials)
                    for pt in range(NDC):
                        o_ps = opsum.tile([PB, QB], F32, tag="psb", bufs=2, name="psb")
                        for c2 in (0, 1):
                            nc.tensor.matmul(
                                o_ps[:],
                                wo_sb[:, ds(D * c2 + PB * pt, PB)],
                                cc[c2][:, ds(QB * qb, QB)],
                                start=(c2 == 0),
                                stop=(c2 == 1),
                            )
                        ob = opool.tile([PB, QB], BF16, tag="ob", bufs=8, name="ob")
                        nc.vector.tensor_copy(ob[:], o_ps[:])
                        nc.sync.dma_start(
                            out_d[:, ds(T * pt + QB * qb, QB)], ob[:]
                        )

    nc.compile()
    return nc


_CACHE = {}


def _get_program(mask2d, has_bias):
    kind, plan, genmask = _mask_plan(mask2d)
    if kind == "generic":
        key = ("generic", mask2d.tobytes(), has_bias)
    else:
        key = (kind, has_bias)
    if key not in _CACHE:
        _CACHE[key] = (_build(plan, len(genmask), has_bias), genmask)
    return _CACHE[key]


def _chunk_major(a, pb=PB):
    """[R, C] (R = k*pb) -> [pb, k*C] laid out chunk-major."""
    r, c = a.shape
    k = r // pb
    return np.ascontiguousarray(
        a.reshape(k, pb, c).transpose(1, 0, 2).reshape(pb, k * c)
    )


def _make_in_maps(x, mask2d, Wq, bq, Wk, bk, Wv, bv, Wo, bo, genmask, has_bias):
    tri = np.triu(np.ones((PB, PB), dtype=np.float32))
    id64 = np.concatenate(
        [np.zeros((HD, HD), np.float32), np.eye(HD, dtype=np.float32)], axis=0
    )

    bf = mybir.dt.np(BF16)

    in_maps = []
    for i in range(NCORES):
        b, g = divmod(i, 4)
        xbT = np.ascontiguousarray(x[b].T)
        wkv = np.concatenate(
            [Wk[:, HD * g:HD * (g + 1)], Wv[:, HD * g:HD * (g + 1)]], axis=1
        )
        m = {
            "xt": _chunk_major(xbT).astype(bf),
            "wq": _chunk_major(Wq[:, GD * g:GD * (g + 1)]).astype(bf),
            "wkv": _chunk_major(wkv).astype(bf),
            "wo": _chunk_major(Wo[GD * g:GD * (g + 1), :]).astype(bf),
            "tri": tri.astype(bf),
            "ident": id64.astype(bf),
        }
        if len(genmask):
            m["genmask"] = genmask.astype(bf)
        if has_bias:
            bq_g = bq[GD * g:GD * (g + 1)]
            m["bqp"] = np.ascontiguousarray(bq_g.reshape(2, PB).T).astype(np.float32)
            m["bkvp"] = np.concatenate(
                [bk[HD * g:HD * (g + 1)], bv[HD * g:HD * (g + 1)]]
            ).reshape(PB, 1).astype(np.float32)
        in_maps.append(m)
    return in_maps


def _assemble(results, bo):
    out = np.empty((B, T, D), dtype=np.float32)
    for b in range(B):
        acc = None
        for g in range(4):
            r = results[4 * b + g]["outT"]          # [128, 8*2048]
            partial = (
                r.astype(np.float32).reshape(PB, NDC, T).transpose(1, 0, 2).reshape(D, T)
            )
            acc = partial if acc is None else acc + partial
        out[b] = acc.T
    if bo is not None:
        out += bo
    return out


def run(inputs, trace=False):
    from concourse.bass_utils import run_bass_kernel_spmd

    x = np.asarray(inputs["x"], dtype=np.float32)
    mask2d = np.asarray(inputs["mask"]).reshape(T, T).astype(bool)
    Wq = np.asarray(inputs["Wq"], np.float32)
    bq = np.asarray(inputs["bq"], np.float32)
    Wk = np.asarray(inputs["Wk"], np.float32)
    bk = np.asarray(inputs["bk"], np.float32)
    Wv = np.asarray(inputs["Wv"], np.float32)
    bv = np.asarray(inputs["bv"], np.float32)
    Wo = np.asarray(inputs["Wo"], np.float32)
    bo = np.asarray(inputs["bo"], np.float32)
    has_bias = bool(bq.any() or bk.any() or bv.any())
    nc, genmask = _get_program(mask2d, has_bias)
    in_maps = _make_in_maps(
        x, mask2d, Wq, bq, Wk, bk, Wv, bv, Wo, bo, genmask, has_bias
    )
    res = run_bass_kernel_spmd(
        nc, in_maps, core_ids=list(range(NCORES)), trace=trace
    )
    return _assemble(res.results, bo if bo.any() else None), res


def kernel(**inputs) -> np.ndarray:
    out, _ = run(inputs, trace=False)
    return out

